# revision 7
# baseline (speedup 1.0000x reference)
"""Bezier stroke renderer on 8 Trainium2 NeuronCores (Bass/Tile SPMD kernel).

Reference semantics: 32 cubic-Bezier strokes, each sampled into a 16-segment
polyline, rasterized onto a 1024x1024 canvas: per pixel and segment,
darkness = clip((2t - dist_to_segment)/(2t), 0, 1), max over segments within a
stroke, then grid = max(grid, darkness * color) over strokes (3 channels).

Strategy (sharding: spatial split of the pixel grid by rows):
  - The canvas is split into 16 blocks of 64 rows; each core owns 2 blocks
    (greedy-balanced by estimated work), giving a [128 partitions x 1024 cols]
    canvas tile per core held in SBUF.
  - Only pixels within 2t+1 of a segment can be painted, so host code builds a
    worklist of (segment, block) windows, chunked into fixed 32-column items.
    All per-item parameters are shipped as per-core data tables; the single
    SPMD instruction stream is identical across cores (counts padded to the
    max over cores).
  - Distance math in the segment's tangent/normal frame, pre-scaled by 1/(2t):
        dist/(2t) = sqrt(relu(a-L)^2 + min(a,0)^2 + b^2)
    where a,b are affine in pixel coords -> computed by TensorE matmuls
    (lhsT = [x_p; 1], per-column coefficients from host tables).
  - Per channel, w_c = (dist/(2t) - 1) * col_c is min-composited into a
    negated-grid accumulator via register-offset dynamic windows (scatter),
    split across the DVE and GPSIMD engines with separate accumulators.
  - Final: out = -min(grid_dve, grid_gps), DMA to DRAM.
"""

import sys
import types
import contextlib
import ctypes

sys.path.insert(0, "/opt/trn_rl_repo")

import numpy as np

G = 1024
P = 16
N = 32
N_CORES = 8
BH = 64           # block height (rows)
NB = G // BH      # 16 blocks
BLOCKS_PER_CORE = NB // N_CORES
W_ITEM = 32       # columns per work item
CHUNK = 512       # packed columns per matmul/PSUM chunk
ITEMS_PER_CHUNK = CHUNK // W_ITEM  # 16

_PROG_CACHE = {}
_HOOK_INSTALLED = False


def _install_ntff_hook():
    """Register the NTFF profile hook (mirrors trn_boot.py) so
    run_bass_kernel_spmd(trace=True) can measure HW exec time."""
    global _HOOK_INSTALLED
    if _HOOK_INSTALLED:
        return
    _HOOK_INSTALLED = True
    try:
        import antenv
        mod = types.ModuleType("antenv.axon_hooks")
        holder = [None]
        mod.set_axon_ntff_profile_hook = lambda h: holder.__setitem__(0, h)
        mod.get_axon_ntff_profile_hook = lambda: holder[0]
        sys.modules["antenv.axon_hooks"] = mod
        antenv.axon_hooks = mod

        lib = ctypes.CDLL("/opt/axon/libaxon_pjrt.so")
        if not hasattr(lib, "axon_start_nrt_profile"):
            return
        lib.axon_start_nrt_profile.argtypes = [
            ctypes.POINTER(ctypes.c_int64),
            ctypes.c_size_t,
        ]
        lib.axon_start_nrt_profile.restype = ctypes.c_int64
        lib.axon_stop_nrt_profile.argtypes = [ctypes.c_char_p]
        lib.axon_stop_nrt_profile.restype = ctypes.c_int64

        @contextlib.contextmanager
        def _hook(output_dir, device_ids):
            import jax
            jax.devices()
            if device_ids:
                ids = (ctypes.c_int64 * len(device_ids))(*device_ids)
                rc = lib.axon_start_nrt_profile(ids, len(device_ids))
            else:
                rc = lib.axon_start_nrt_profile(None, 0)
            if rc != 0:
                raise RuntimeError(f"axon_start_nrt_profile rc={rc}")
            try:
                yield
            finally:
                n = lib.axon_stop_nrt_profile(str(output_dir).encode())
                print(f"profile: {n} file(s) written to {output_dir}",
                      file=sys.stderr)

        mod.set_axon_ntff_profile_hook(_hook)
    except Exception:
        pass


# ---------------------------------------------------------------- host side

def _bezier_weights_f32(p):
    t = np.arange(p, dtype=np.float64)
    w1 = (p - t) ** 3 / p ** 3
    w2 = 3 * (p - t) ** 2 * t / p ** 3
    w3 = 3 * (p - t) * t ** 2 / p ** 3
    w4 = t ** 3 / p ** 3
    return np.stack([w1, w2, w3, w4]).astype(np.float32)  # (4, P)


def _polylines(strokes):
    """(N,2,4) f32 -> (N, P+1, 2) f32 polyline points in pixel units,
    mirroring reference.curve_to_stroke in float32."""
    W = _bezier_weights_f32(P)
    s = strokes.astype(np.float32)
    pts, derivs = s[:, :, :2], s[:, :, 2:]
    before = pts - derivs
    after = pts + derivs
    p1, p2, p3, p4 = pts[:, :-1], after[:, :-1], before[:, 1:], pts[:, 1:]
    cp = np.stack([p1, p2, p3, p4], axis=3)          # (N, 1, 2, 4)
    sp = np.einsum("nsdk,kp->nspd", cp, W).astype(np.float32)  # (N,1,P,2)
    sp = sp.reshape(s.shape[0], -1, 2)
    poly = np.concatenate([sp, pts[:, -1:, :]], axis=1).astype(np.float32)
    return poly * np.float32(G)


def _band_clip(v, w, pad, x0, x1):
    """Clip segment v->w (f64) to row band [x0-pad, x1+pad]; return padded,
    canvas-clamped column range [c0, c1] or None."""
    lo_x, hi_x = x0 - pad, x1 + pad
    dx = w[0] - v[0]
    if abs(dx) < 1e-12:
        if v[0] < lo_x or v[0] > hi_x:
            return None
        s0, s1 = 0.0, 1.0
    else:
        sa = (lo_x - v[0]) / dx
        sb = (hi_x - v[0]) / dx
        s0 = max(0.0, min(sa, sb))
        s1 = min(1.0, max(sa, sb))
        if s0 > s1:
            return None
    ya = v[1] + s0 * (w[1] - v[1])
    yb = v[1] + s1 * (w[1] - v[1])
    c0 = max(0.0, min(ya, yb) - pad)
    c1 = min(G - 1.0, max(ya, yb) + pad)
    if c1 < c0:
        return None
    return int(np.floor(c0)), int(np.ceil(c1))


def _build_worklists(strokes, thicknesses, colors):
    """Returns (blocks_of_core, items_per_core, t, col) where each
    items_per_core[c] is a list of (n, v(2,), w(2,), c0)."""
    poly = _polylines(strokes).astype(np.float64)          # (N, P+1, 2)
    t = np.maximum(thicknesses.astype(np.float32) * np.float32(2.0)
                   + np.float32(0.5), np.float32(0.5))[:, 0]  # f32 (N,)
    col = np.clip(colors.astype(np.float32), 0.0, 1.0)     # (N, 3)
    r = 2.0 * t.astype(np.float64)
    pad = r + 1.0

    items_by_block = [[] for _ in range(NB)]
    cost = np.zeros(NB)
    for n in range(N):
        for i in range(P):
            v = poly[n, i]
            w = poly[n, i + 1]
            for b in range(NB):
                clip = _band_clip(v, w, pad[n], BH * b, BH * b + BH - 1)
                if clip is None:
                    continue
                c0, c1 = clip
                cstart = c0
                while cstart <= c1:
                    cc = min(cstart, G - W_ITEM)
                    items_by_block[b].append((n, v, w, cc))
                    cstart += W_ITEM
                cost[b] += c1 - c0 + 1

    order = np.argsort(-cost)
    loads = np.zeros(N_CORES)
    blocks_of = [[] for _ in range(N_CORES)]
    for b in order:
        cands = [c for c in range(N_CORES) if len(blocks_of[c]) < BLOCKS_PER_CORE]
        c = min(cands, key=lambda c: loads[c])
        blocks_of[c].append(int(b))
        loads[c] += cost[b]
    for c in range(N_CORES):
        blocks_of[c].sort()

    items_per_core = [
        [it for b in blocks_of[c] for it in items_by_block[b]]
        for c in range(N_CORES)
    ]
    return blocks_of, items_per_core, t, col


def _build_tables(blocks_of, items_per_core, t, col, nitems):
    """Build per-core input tables. Returns list of dicts (one per core)."""
    in_maps = []
    for c in range(N_CORES):
        items = items_per_core[c]
        k = len(items)
        vx = np.zeros(nitems); vy = np.zeros(nitems)
        wx = np.zeros(nitems); wy = np.zeros(nitems)
        c0 = np.zeros(nitems, np.int64)
        i2t = np.full(nitems, 1.0)
        cols = np.zeros((nitems, 3))
        valid = np.zeros(nitems, bool)
        for j, (n, v, w, cc) in enumerate(items):
            vx[j], vy[j] = v
            wx[j], wy[j] = w
            c0[j] = cc
            i2t[j] = 1.0 / (2.0 * np.float64(t[n]))
            cols[j] = col[n]
            valid[j] = True

        dx = wx - vx
        dy = wy - vy
        L = np.hypot(dx, dy)
        safe = L > 1e-9
        taux = np.where(safe, dx / np.where(safe, L, 1.0), 1.0)
        tauy = np.where(safe, dy / np.where(safe, L, 1.0), 0.0)
        Leff = np.where(safe, L, 0.0)
        nux = -tauy
        nuy = taux

        av = vx * taux + vy * tauy
        bv = vx * nux + vy * nuy
        ycols = c0[:, None] + np.arange(W_ITEM)[None, :]     # (nitems, 32)
        a1 = taux * i2t
        b1 = nux * i2t
        a2 = (ycols * tauy[:, None] - av[:, None]) * i2t[:, None]
        a2l = a2 - (Leff * i2t)[:, None]
        b2 = (ycols * nuy[:, None] - bv[:, None]) * i2t[:, None]

        dead = ~valid
        a1[dead] = 0.0; b1[dead] = 0.0
        a2[dead] = 0.0; a2l[dead] = 0.0; b2[dead] = 0.0
        cols[dead] = 0.0
        c0[dead] = 0

        packw = nitems * W_ITEM
        # rows are (const_coeff, x_coeff) pairs: lhsT rows are (ones, x)
        rt = np.zeros((6, packw), np.float32)
        rt[0] = a2.ravel().astype(np.float32)
        rt[1] = np.repeat(a1, W_ITEM).astype(np.float32)
        rt[2] = a2l.ravel().astype(np.float32)
        rt[3] = rt[1]
        rt[4] = b2.ravel().astype(np.float32)
        rt[5] = np.repeat(b1, W_ITEM).astype(np.float32)
        rc = np.stack([
            np.repeat(cols[:, 0], W_ITEM),
            np.repeat(cols[:, 1], W_ITEM),
            np.repeat(cols[:, 2], W_ITEM),
        ]).astype(np.float32)

        off = (3 * c0).astype(np.int32).reshape(1, nitems)

        xs = np.zeros(128, np.float32)
        for half, b in enumerate(blocks_of[c]):
            xs[half * BH:(half + 1) * BH] = BH * b + np.arange(BH)
        xt = np.zeros((66, 128), np.float32)
        for base in (0, 32, 64):
            xt[base] = 1.0
            xt[base + 1] = xs

        in_maps.append({"xt": xt, "rt": rt, "rc": rc, "off": off})
    return in_maps


# ---------------------------------------------------------------- bass side

def _build_program(nitems):
    import concourse.bacc as bacc
    import concourse.mybir as mybir
    import concourse.bass as bass
    from concourse import tile

    f32 = mybir.dt.float32
    packw = nitems * W_ITEM
    nchunks = packw // CHUNK
    assert nchunks * CHUNK == packw

    nc = bacc.Bacc("TRN2", target_bir_lowering=False, debug=False,
                   num_devices=N_CORES)
    xt_d = nc.dram_tensor("xt", [66, 128], f32, kind="ExternalInput").ap()
    rt_d = nc.dram_tensor("rt", [6, packw], f32, kind="ExternalInput").ap()
    rc_d = nc.dram_tensor("rc", [3, packw], f32, kind="ExternalInput").ap()
    off_d = nc.dram_tensor("off", [1, nitems], mybir.dt.int32,
                           kind="ExternalInput").ap()
    out_d = nc.dram_tensor("out", [128, 3 * G], f32, kind="ExternalOutput").ap()

    AF = mybir.ActivationFunctionType
    OP = mybir.AluOpType

    with tile.TileContext(nc) as tc:
        with (
            tc.tile_pool(name="const", bufs=1) as constp,
            tc.tile_pool(name="work", bufs=3) as workp,
            tc.tile_pool(name="psum", bufs=8, space="PSUM") as psump,
        ):
            # matmul operand pairs must sit at base partitions 0/32/64,
            # matching between lhsT and rhs
            xt = constp.tile([66, 128], f32)
            nc.sync.dma_start(xt[:], xt_d[:])
            rt = constp.tile([66, packw], f32)
            nc.sync.dma_start(rt[0:2, :], rt_d[0:2, :])
            nc.sync.dma_start(rt[32:34, :], rt_d[2:4, :])
            nc.sync.dma_start(rt[64:66, :], rt_d[4:6, :])
            rc = constp.tile([65, packw], f32)
            nc.sync.dma_start(rc[0:1, :], rc_d[0:1, :])
            nc.sync.dma_start(rc[32:33, :], rc_d[1:2, :])
            nc.sync.dma_start(rc[64:65, :], rc_d[2:3, :])
            off = constp.tile([1, nitems], mybir.dt.int32)
            nc.sync.dma_start(off[:], off_d[:])

            grid_dve = constp.tile([128, 3 * G], f32)
            nc.vector.memset(grid_dve[:], 0.0)

            vint = constp.tile([128, 3 * packw], f32)
            vint3 = vint[:].rearrange("p (n c) -> p n c", c=3)

            for ch in range(nchunks):
                sl = slice(ch * CHUNK, (ch + 1) * CHUNK)
                pa = psump.tile([128, CHUNK], f32, tag="ps")
                pa2 = psump.tile([128, CHUNK], f32, tag="ps")
                pb = psump.tile([128, CHUNK], f32, tag="ps")
                pc0 = psump.tile([128, CHUNK], f32, tag="ps")
                pc1 = psump.tile([128, CHUNK], f32, tag="ps")
                pc2 = psump.tile([128, CHUNK], f32, tag="ps")

                nc.tensor.matmul(pa[:], xt[0:2, :], rt[0:2, sl])
                nc.tensor.matmul(pa2[:], xt[32:34, :], rt[32:34, sl])
                nc.tensor.matmul(pb[:], xt[64:66, :], rt[64:66, sl])
                nc.tensor.matmul(pc0[:], xt[0:1, :], rc[0:1, sl])
                nc.tensor.matmul(pc1[:], xt[32:33, :], rc[32:33, sl])
                nc.tensor.matmul(pc2[:], xt[64:65, :], rc[64:65, sl])

                q1 = workp.tile([128, CHUNK], f32, tag="q1")
                q2 = workp.tile([128, CHUNK], f32, tag="q2")
                s1 = workp.tile([128, CHUNK], f32, tag="s1")
                s2 = workp.tile([128, CHUNK], f32, tag="s2")
                d2a = workp.tile([128, CHUNK], f32, tag="d2a")
                d2 = workp.tile([128, CHUNK], f32, tag="d2")
                dd = workp.tile([128, CHUNK], f32, tag="dd")

                # overshoot beyond segment end / before start, in 2t units
                nc.scalar.activation(q1[:], pa2[:], AF.Relu)
                nc.scalar.activation(q2[:], pa[:], AF.Relu, scale=-1.0)
                nc.scalar.activation(s1[:], q1[:], AF.Square)
                nc.scalar.activation(s2[:], q2[:], AF.Square)
                nc.vector.tensor_tensor(d2a[:], s1[:], s2[:], op=OP.add)
                # + normal distance^2 (b^2) -- square on ACT, add on DVE
                sb = workp.tile([128, CHUNK], f32, tag="sb")
                nc.scalar.activation(sb[:], pb[:], AF.Square)
                nc.vector.tensor_tensor(d2[:], d2a[:], sb[:], op=OP.add)
                nc.scalar.activation(dd[:], d2[:], AF.Sqrt)

                # w_c = (dd - 1) * col_c, interleaved by channel
                vch = vint3[:, sl, :]
                nc.vector.scalar_tensor_tensor(
                    vch[:, :, 0], dd[:], 1.0, pc0[:], op0=OP.subtract, op1=OP.mult)
                nc.vector.scalar_tensor_tensor(
                    vch[:, :, 1], dd[:], 1.0, pc1[:], op0=OP.subtract, op1=OP.mult)
                nc.vector.scalar_tensor_tensor(
                    vch[:, :, 2], dd[:], 1.0, pc2[:], op0=OP.subtract, op1=OP.mult)

            # scatter: min-composite each item window into the canvas
            BATCH = 8
            for base in range(0, nitems, BATCH):
                cnt = min(BATCH, nitems - base)
                _, vals = nc.values_load_multi_w_load_instructions(
                    off[0:1, base:base + cnt],
                    engines=[nc.vector.engine],
                    min_val=0,
                    max_val=3 * (G - W_ITEM),
                    skip_runtime_bounds_check=True,
                )
                for j, val in enumerate(vals):
                    k = base + j
                    dst = grid_dve[:, bass.ds(val, 3 * W_ITEM)]
                    src = vint[:, 3 * W_ITEM * k: 3 * W_ITEM * (k + 1)]
                    nc.vector.tensor_tensor(dst, dst, src, op=OP.min)

            # negate + store
            outt = constp.tile([128, 3 * G], f32)
            for piece in range(4):
                slp = slice(piece * 3 * G // 4, (piece + 1) * 3 * G // 4)
                nc.scalar.activation(outt[:, slp], grid_dve[:, slp],
                                     AF.Copy, scale=-1.0)
                nc.sync.dma_start(out_d[:, slp], outt[:, slp])

    nc.compile()
    return nc


# ---------------------------------------------------------------- entry

def _prepare(strokes, thicknesses, colors):
    blocks_of, items_per_core, t, col = _build_worklists(
        strokes, thicknesses, colors)
    max_items = max(len(it) for it in items_per_core)
    nitems = ((max_items + ITEMS_PER_CHUNK - 1) // ITEMS_PER_CHUNK) \
        * ITEMS_PER_CHUNK
    nitems = max(nitems, ITEMS_PER_CHUNK)
    in_maps = _build_tables(blocks_of, items_per_core, t, col, nitems)
    return blocks_of, in_maps, nitems


def kernel(strokes, thicknesses, colors):
    _install_ntff_hook()
    from concourse.bass_utils import run_bass_kernel_spmd

    strokes = np.asarray(strokes)
    thicknesses = np.asarray(thicknesses)
    colors = np.asarray(colors)

    blocks_of, in_maps, nitems = _prepare(strokes, thicknesses, colors)
    if nitems not in _PROG_CACHE:
        _PROG_CACHE[nitems] = _build_program(nitems)
    nc = _PROG_CACHE[nitems]

    res = run_bass_kernel_spmd(nc, in_maps, list(range(N_CORES)))

    out = np.zeros((3, G, G), np.float32)
    for c in range(N_CORES):
        o = res.results[c]["out"].reshape(128, G, 3)
        for half, b in enumerate(blocks_of[c]):
            rows = o[half * BH:(half + 1) * BH]          # (64, 1024, 3)
            out[:, BH * b:BH * (b + 1), :] = rows.transpose(2, 0, 1)
    return out


if __name__ == "__main__":
    rng = np.random.default_rng(0)
    s = rng.random((N, 2, 4), np.float32)
    th = rng.random((N, 1), np.float32)
    co = rng.random((N, 3), np.float32)
    g = kernel(s, th, co)
    print("out", g.shape, g.dtype, g.min(), g.max())


# revision 14
# speedup vs baseline: 1.0409x; 1.0409x over previous
"""Bezier stroke renderer on 8 Trainium2 NeuronCores (Bass/Tile SPMD kernel).

Reference semantics: 32 cubic-Bezier strokes, each sampled into a 16-segment
polyline, rasterized onto a 1024x1024 canvas: per pixel and segment,
darkness = clip((2t - dist_to_segment)/(2t), 0, 1), max over segments within a
stroke, then grid = max(grid, darkness * color) over strokes (3 channels).

Strategy (sharding: spatial split of the pixel grid by rows):
  - The canvas is split into 16 blocks of 64 rows; each core owns 2 blocks
    (greedy-balanced by estimated work), giving a [128 partitions x 1024 cols]
    canvas tile per core held in SBUF.
  - Only pixels within 2t+1 of a segment can be painted, so host code builds a
    worklist of (segment, block) windows, chunked into fixed 32-column items.
    All per-item parameters are shipped as per-core data tables; the single
    SPMD instruction stream is identical across cores (counts padded to the
    max over cores).
  - Distance math in the segment's tangent/normal frame, pre-scaled by 1/(2t):
        dist/(2t) = sqrt(relu(a-L)^2 + min(a,0)^2 + b^2)
    where a,b are affine in pixel coords -> computed by TensorE matmuls
    (lhsT = [x_p; 1], per-column coefficients from host tables).
  - Per channel, w_c = (dist/(2t) - 1) * col_c is min-composited into a
    negated-grid accumulator via register-offset dynamic windows (scatter),
    split across the DVE and GPSIMD engines with separate accumulators.
  - Final: out = -min(grid_dve, grid_gps), DMA to DRAM.
"""

import sys
import types
import contextlib
import ctypes

sys.path.insert(0, "/opt/trn_rl_repo")

import numpy as np

G = 1024
P = 16
N = 32
N_CORES = 8
BH = 64           # block height (rows)
NB = G // BH      # 16 blocks
BLOCKS_PER_CORE = NB // N_CORES
W_ITEM = 32       # columns per work item
CHUNK = 512       # packed columns per matmul/PSUM chunk
ITEMS_PER_CHUNK = CHUNK // W_ITEM  # 16

_PROG_CACHE = {}
_HOOK_INSTALLED = False


def _install_ntff_hook():
    """Register the NTFF profile hook (mirrors trn_boot.py) so
    run_bass_kernel_spmd(trace=True) can measure HW exec time."""
    global _HOOK_INSTALLED
    if _HOOK_INSTALLED:
        return
    _HOOK_INSTALLED = True
    try:
        import antenv
        mod = types.ModuleType("antenv.axon_hooks")
        holder = [None]
        mod.set_axon_ntff_profile_hook = lambda h: holder.__setitem__(0, h)
        mod.get_axon_ntff_profile_hook = lambda: holder[0]
        sys.modules["antenv.axon_hooks"] = mod
        antenv.axon_hooks = mod

        lib = ctypes.CDLL("/opt/axon/libaxon_pjrt.so")
        if not hasattr(lib, "axon_start_nrt_profile"):
            return
        lib.axon_start_nrt_profile.argtypes = [
            ctypes.POINTER(ctypes.c_int64),
            ctypes.c_size_t,
        ]
        lib.axon_start_nrt_profile.restype = ctypes.c_int64
        lib.axon_stop_nrt_profile.argtypes = [ctypes.c_char_p]
        lib.axon_stop_nrt_profile.restype = ctypes.c_int64

        @contextlib.contextmanager
        def _hook(output_dir, device_ids):
            import jax
            jax.devices()
            if device_ids:
                ids = (ctypes.c_int64 * len(device_ids))(*device_ids)
                rc = lib.axon_start_nrt_profile(ids, len(device_ids))
            else:
                rc = lib.axon_start_nrt_profile(None, 0)
            if rc != 0:
                raise RuntimeError(f"axon_start_nrt_profile rc={rc}")
            try:
                yield
            finally:
                n = lib.axon_stop_nrt_profile(str(output_dir).encode())
                print(f"profile: {n} file(s) written to {output_dir}",
                      file=sys.stderr)

        mod.set_axon_ntff_profile_hook(_hook)
    except Exception:
        pass


# ---------------------------------------------------------------- host side

def _bezier_weights_f32(p):
    t = np.arange(p, dtype=np.float64)
    w1 = (p - t) ** 3 / p ** 3
    w2 = 3 * (p - t) ** 2 * t / p ** 3
    w3 = 3 * (p - t) * t ** 2 / p ** 3
    w4 = t ** 3 / p ** 3
    return np.stack([w1, w2, w3, w4]).astype(np.float32)  # (4, P)


def _polylines(strokes):
    """(N,2,4) f32 -> (N, P+1, 2) f32 polyline points in pixel units,
    mirroring reference.curve_to_stroke in float32."""
    W = _bezier_weights_f32(P)
    s = strokes.astype(np.float32)
    pts, derivs = s[:, :, :2], s[:, :, 2:]
    before = pts - derivs
    after = pts + derivs
    p1, p2, p3, p4 = pts[:, :-1], after[:, :-1], before[:, 1:], pts[:, 1:]
    cp = np.stack([p1, p2, p3, p4], axis=3)          # (N, 1, 2, 4)
    sp = np.einsum("nsdk,kp->nspd", cp, W).astype(np.float32)  # (N,1,P,2)
    sp = sp.reshape(s.shape[0], -1, 2)
    poly = np.concatenate([sp, pts[:, -1:, :]], axis=1).astype(np.float32)
    return poly * np.float32(G)


def _band_clip(v, w, pad, x0, x1):
    """Clip segment v->w (f64) to row band [x0-pad, x1+pad]; return padded,
    canvas-clamped column range [c0, c1] or None."""
    lo_x, hi_x = x0 - pad, x1 + pad
    dx = w[0] - v[0]
    if abs(dx) < 1e-12:
        if v[0] < lo_x or v[0] > hi_x:
            return None
        s0, s1 = 0.0, 1.0
    else:
        sa = (lo_x - v[0]) / dx
        sb = (hi_x - v[0]) / dx
        s0 = max(0.0, min(sa, sb))
        s1 = min(1.0, max(sa, sb))
        if s0 > s1:
            return None
    ya = v[1] + s0 * (w[1] - v[1])
    yb = v[1] + s1 * (w[1] - v[1])
    c0 = max(0.0, min(ya, yb) - pad)
    c1 = min(G - 1.0, max(ya, yb) + pad)
    if c1 < c0:
        return None
    return int(np.floor(c0)), int(np.ceil(c1))


def _build_worklists(strokes, thicknesses, colors):
    """Returns (blocks_of_core, items_per_core, t, col) where each
    items_per_core[c] is a list of (n, v(2,), w(2,), c0)."""
    poly = _polylines(strokes).astype(np.float64)          # (N, P+1, 2)
    t = np.maximum(thicknesses.astype(np.float32) * np.float32(2.0)
                   + np.float32(0.5), np.float32(0.5))[:, 0]  # f32 (N,)
    col = np.clip(colors.astype(np.float32), 0.0, 1.0)     # (N, 3)
    r = 2.0 * t.astype(np.float64)
    pad = r + 1.0

    items_by_block = [[] for _ in range(NB)]
    cost = np.zeros(NB)
    for n in range(N):
        for i in range(P):
            v = poly[n, i]
            w = poly[n, i + 1]
            for b in range(NB):
                clip = _band_clip(v, w, pad[n], BH * b, BH * b + BH - 1)
                if clip is None:
                    continue
                c0, c1 = clip
                cstart = c0
                while cstart <= c1:
                    cc = min(cstart, G - W_ITEM)
                    items_by_block[b].append((n, v, w, cc))
                    cstart += W_ITEM
                    cost[b] += 1.0

    order = np.argsort(-cost)
    loads = np.zeros(N_CORES)
    blocks_of = [[] for _ in range(N_CORES)]
    for b in order:
        cands = [c for c in range(N_CORES) if len(blocks_of[c]) < BLOCKS_PER_CORE]
        c = min(cands, key=lambda c: loads[c])
        blocks_of[c].append(int(b))
        loads[c] += cost[b]
    for c in range(N_CORES):
        blocks_of[c].sort()

    items_per_core = [
        [it for b in blocks_of[c] for it in items_by_block[b]]
        for c in range(N_CORES)
    ]
    return blocks_of, items_per_core, t, col


def _build_tables(blocks_of, items_per_core, t, col, nitems):
    """Build per-core input tables. Returns list of dicts (one per core)."""
    in_maps = []
    for c in range(N_CORES):
        items = items_per_core[c]
        k = len(items)
        vx = np.zeros(nitems); vy = np.zeros(nitems)
        wx = np.zeros(nitems); wy = np.zeros(nitems)
        c0 = np.zeros(nitems, np.int64)
        i2t = np.full(nitems, 1.0)
        cols = np.zeros((nitems, 3))
        valid = np.zeros(nitems, bool)
        for j, (n, v, w, cc) in enumerate(items):
            vx[j], vy[j] = v
            wx[j], wy[j] = w
            c0[j] = cc
            i2t[j] = 1.0 / (2.0 * np.float64(t[n]))
            cols[j] = col[n]
            valid[j] = True

        dx = wx - vx
        dy = wy - vy
        L = np.hypot(dx, dy)
        safe = L > 1e-9
        taux = np.where(safe, dx / np.where(safe, L, 1.0), 1.0)
        tauy = np.where(safe, dy / np.where(safe, L, 1.0), 0.0)
        Leff = np.where(safe, L, 0.0)
        nux = -tauy
        nuy = taux

        av = vx * taux + vy * tauy
        bv = vx * nux + vy * nuy
        ycols = c0[:, None] + np.arange(W_ITEM)[None, :]     # (nitems, 32)
        a1 = taux * i2t
        b1 = nux * i2t
        a2 = (ycols * tauy[:, None] - av[:, None]) * i2t[:, None]
        a2l = a2 - (Leff * i2t)[:, None]
        b2 = (ycols * nuy[:, None] - bv[:, None]) * i2t[:, None]

        dead = ~valid
        a1[dead] = 0.0; b1[dead] = 0.0
        a2[dead] = 0.0; a2l[dead] = 0.0; b2[dead] = 0.0
        cols[dead] = 0.0
        c0[dead] = 0

        packw = nitems * W_ITEM
        # rows are (const_coeff, x_coeff) pairs: lhsT rows are (ones, x)
        rt = np.zeros((6, packw), np.float32)
        rt[0] = a2.ravel().astype(np.float32)
        rt[1] = np.repeat(a1, W_ITEM).astype(np.float32)
        rt[2] = a2l.ravel().astype(np.float32)
        rt[3] = rt[1]
        rt[4] = b2.ravel().astype(np.float32)
        rt[5] = np.repeat(b1, W_ITEM).astype(np.float32)
        rc = cols.T.astype(np.float32).copy()     # (3, nitems), per item

        off = c0.astype(np.int32).reshape(1, nitems)

        xs = np.zeros(128, np.float32)
        for half, b in enumerate(blocks_of[c]):
            xs[half * BH:(half + 1) * BH] = BH * b + np.arange(BH)
        xt = np.zeros((66, 128), np.float32)
        for base in (0, 32, 64):
            xt[base] = 1.0
            xt[base + 1] = xs

        in_maps.append({"xt": xt, "rt": rt, "rc": rc, "off": off})
    return in_maps


# ---------------------------------------------------------------- bass side

def _build_program(nitems):
    import concourse.bacc as bacc
    import concourse.mybir as mybir
    import concourse.bass as bass
    from concourse import tile

    f32 = mybir.dt.float32
    packw = nitems * W_ITEM
    nchunks = packw // CHUNK
    assert nchunks * CHUNK == packw

    nc = bacc.Bacc("TRN2", target_bir_lowering=False, debug=False,
                   num_devices=N_CORES)
    xt_d = nc.dram_tensor("xt", [66, 128], f32, kind="ExternalInput").ap()
    rt_d = nc.dram_tensor("rt", [6, packw], f32, kind="ExternalInput").ap()
    rc_d = nc.dram_tensor("rc", [3, nitems], f32, kind="ExternalInput").ap()
    off_d = nc.dram_tensor("off", [1, nitems], mybir.dt.int32,
                           kind="ExternalInput").ap()
    out_d = nc.dram_tensor("out", [128, 3 * G], f32, kind="ExternalOutput").ap()

    AF = mybir.ActivationFunctionType
    OP = mybir.AluOpType
    NACC = 4  # scatter accumulators (breaks the RMW dependency chain)

    with tile.TileContext(nc) as tc:
        with (
            tc.tile_pool(name="const", bufs=1) as constp,
            tc.tile_pool(name="work", bufs=2) as workp,
            tc.tile_pool(name="psum", bufs=8, space="PSUM") as psump,
        ):
            # matmul operand pairs must sit at base partitions 0/32/64,
            # matching between lhsT and rhs
            xt = constp.tile([66, 128], f32)
            nc.sync.dma_start(xt[:], xt_d[:])
            rt = constp.tile([66, packw], f32)
            nc.sync.dma_start(rt[0:2, :], rt_d[0:2, :])
            nc.sync.dma_start(rt[32:34, :], rt_d[2:4, :])
            nc.sync.dma_start(rt[64:66, :], rt_d[4:6, :])
            rc = constp.tile([65, nitems], f32)
            nc.sync.dma_start(rc[0:1, :], rc_d[0:1, :])
            nc.sync.dma_start(rc[32:33, :], rc_d[1:2, :])
            nc.sync.dma_start(rc[64:65, :], rc_d[2:3, :])
            off = constp.tile([1, nitems], mybir.dt.int32)
            nc.sync.dma_start(off[:], off_d[:])

            # per-item color broadcast tables [128, nitems], one per channel
            coltab = []
            for c, base in ((0, 0), (1, 32), (2, 64)):
                ct = constp.tile([128, nitems], f32, tag=f"coltab{c}")
                for co in range(0, nitems, CHUNK):
                    ce = min(co + CHUNK, nitems)
                    pc = psump.tile([128, ce - co], f32, tag="ps")
                    nc.tensor.matmul(pc[:], xt[base:base + 1, :],
                                     rc[base:base + 1, co:ce])
                    nc.scalar.copy(ct[:, co:ce], pc[:])
                coltab.append(ct)

            grids = []
            for a in range(NACC):
                g = constp.tile([128, 3 * G], f32, tag=f"grid{a}")
                if a == 0:
                    nc.vector.memset(g[:], 0.0)
                else:
                    nc.gpsimd.memset(g[:], 0.0)
                grids.append(g)

            # vint: channel-planar packed values, vint[:, c*packw + j]
            vint = constp.tile([128, 3 * packw], f32)

            for ch in range(nchunks):
                sl = slice(ch * CHUNK, (ch + 1) * CHUNK)
                k0 = ch * ITEMS_PER_CHUNK
                k1 = (ch + 1) * ITEMS_PER_CHUNK
                pa = psump.tile([128, CHUNK], f32, tag="ps")
                pa2 = psump.tile([128, CHUNK], f32, tag="ps")
                pb = psump.tile([128, CHUNK], f32, tag="ps")

                nc.tensor.matmul(pa[:], xt[0:2, :], rt[0:2, sl])
                nc.tensor.matmul(pa2[:], xt[32:34, :], rt[32:34, sl])
                nc.tensor.matmul(pb[:], xt[64:66, :], rt[64:66, sl])

                q1 = workp.tile([128, CHUNK], f32, tag="q1")
                q2 = workp.tile([128, CHUNK], f32, tag="q2")
                s1 = workp.tile([128, CHUNK], f32, tag="s1")
                s2 = workp.tile([128, CHUNK], f32, tag="s2")
                d2a = workp.tile([128, CHUNK], f32, tag="d2a")
                d2 = workp.tile([128, CHUNK], f32, tag="d2")
                dd = workp.tile([128, CHUNK], f32, tag="dd")
                sb = workp.tile([128, CHUNK], f32, tag="sb")

                # overshoot beyond segment end / before start, in 2t units
                nc.scalar.activation(q1[:], pa2[:], AF.Relu)
                nc.scalar.activation(q2[:], pa[:], AF.Relu, scale=-1.0)
                nc.scalar.activation(s1[:], q1[:], AF.Square)
                nc.scalar.activation(s2[:], q2[:], AF.Square)
                nc.scalar.activation(sb[:], pb[:], AF.Square)
                nc.gpsimd.tensor_tensor(d2a[:], s1[:], s2[:], op=OP.add)
                nc.gpsimd.tensor_tensor(d2[:], d2a[:], sb[:], op=OP.add)
                nc.scalar.activation(dd[:], d2[:], AF.Sqrt)

                # w_c = (dd - 1) * col_c into the channel-planar vint
                dd3 = dd[:].rearrange("p (k r) -> p k r", r=W_ITEM)
                for c in range(3):
                    vdst = vint[:, c * packw + ch * CHUNK:
                                c * packw + (ch + 1) * CHUNK].rearrange(
                        "p (k r) -> p k r", r=W_ITEM)
                    cexp = coltab[c][:, k0:k1].to_broadcast(
                        (128, ITEMS_PER_CHUNK, W_ITEM))
                    nc.vector.scalar_tensor_tensor(
                        vdst, dd3, 1.0, cexp, op0=OP.subtract, op1=OP.mult)

            # scatter: min-composite each item window into the canvas
            # (rotating over NACC accumulators to keep the DVE pipe full)
            grids3 = [g[:].rearrange("p (c y) -> p c y", c=3) for g in grids]
            vint3 = vint[:].rearrange("p (c j) -> p c j", c=3)
            BATCH = 16
            for base in range(0, nitems, BATCH):
                cnt = min(BATCH, nitems - base)
                _, vals = nc.values_load_multi_w_load_instructions(
                    off[0:1, base:base + cnt],
                    engines=[nc.vector.engine],
                    min_val=0,
                    max_val=G - W_ITEM,
                    skip_runtime_bounds_check=True,
                )
                for j, val in enumerate(vals):
                    k = base + j
                    dst = grids3[k % NACC][:, :, bass.ds(val, W_ITEM)]
                    src = vint3[:, :, W_ITEM * k: W_ITEM * (k + 1)]
                    nc.vector.tensor_tensor(dst, dst, src, op=OP.min)

            # merge accumulators, negate, store
            m01 = grids[0]
            nc.vector.tensor_tensor(m01[:], grids[0][:], grids[1][:], op=OP.min)
            m23 = grids[2]
            nc.vector.tensor_tensor(m23[:], grids[2][:], grids[3][:], op=OP.min)
            outt = constp.tile([128, 3 * G], f32)
            for piece in range(4):
                slp = slice(piece * 3 * G // 4, (piece + 1) * 3 * G // 4)
                nc.vector.tensor_tensor(m01[:, slp], m01[:, slp], m23[:, slp],
                                        op=OP.min)
                nc.scalar.activation(outt[:, slp], m01[:, slp],
                                     AF.Copy, scale=-1.0)
                nc.sync.dma_start(out_d[:, slp], outt[:, slp])

    nc.compile()
    return nc


# ---------------------------------------------------------------- entry

def _prepare(strokes, thicknesses, colors):
    blocks_of, items_per_core, t, col = _build_worklists(
        strokes, thicknesses, colors)
    max_items = max(len(it) for it in items_per_core)
    nitems = ((max_items + ITEMS_PER_CHUNK - 1) // ITEMS_PER_CHUNK) \
        * ITEMS_PER_CHUNK
    nitems = max(nitems, ITEMS_PER_CHUNK)
    in_maps = _build_tables(blocks_of, items_per_core, t, col, nitems)
    return blocks_of, in_maps, nitems


def kernel(strokes, thicknesses, colors):
    _install_ntff_hook()
    from concourse.bass_utils import run_bass_kernel_spmd

    strokes = np.asarray(strokes)
    thicknesses = np.asarray(thicknesses)
    colors = np.asarray(colors)

    blocks_of, in_maps, nitems = _prepare(strokes, thicknesses, colors)
    if nitems not in _PROG_CACHE:
        _PROG_CACHE[nitems] = _build_program(nitems)
    nc = _PROG_CACHE[nitems]

    res = run_bass_kernel_spmd(nc, in_maps, list(range(N_CORES)))

    out = np.zeros((3, G, G), np.float32)
    for c in range(N_CORES):
        o = res.results[c]["out"].reshape(128, 3, G)     # channel-planar
        for half, b in enumerate(blocks_of[c]):
            rows = o[half * BH:(half + 1) * BH]          # (64, 3, 1024)
            out[:, BH * b:BH * (b + 1), :] = rows.transpose(1, 0, 2)
    return out


if __name__ == "__main__":
    rng = np.random.default_rng(0)
    s = rng.random((N, 2, 4), np.float32)
    th = rng.random((N, 1), np.float32)
    co = rng.random((N, 3), np.float32)
    g = kernel(s, th, co)
    print("out", g.shape, g.dtype, g.min(), g.max())
